# revision 19
# baseline (speedup 1.0000x reference)
"""CALayer (channel attention) Trainium2 kernel.

x: [16, 64, 256, 256] f32.  y = sigmoid(mlp(avgpool(x)) + mlp(maxpool(x)));
out = x * y[:, :, None, None].

Sharding: data-parallel over batch, 2 samples per core on 8 cores. Per-core
view is [(2*64)=128 partitions, 65536 spatial].  Pass 1 streams 32 chunks of
[128, 2048], computing the spatial sum on ScalarE (activation-Copy accum_out)
and the max on VectorE; the first NCACHE chunks stay resident in SBUF.
Pass 2 runs the 64->16->64 MLP on TensorE.  Pass 3 rescales each chunk by the
per-(b,c) sigmoid output and DMAs it out; only the uncached chunks re-read HBM.
"""

import contextlib

import numpy as np

B, C, H, W = 16, 64, 256, 256
S = H * W            # 65536 spatial elements per (b, c)
RED = 16             # MLP hidden dim
NCORES = 8
BPC = B // NCORES    # batch per core = 2
P = BPC * C          # 128 partitions per core

F = 2048             # cached-chunk free-dim size (1 MiB per [128, F] f32 DMA)
NCACHE = 26          # chunks kept resident in SBUF between pass 1 and pass 3
FS = 1024            # streamed-chunk free-dim size (smaller -> more cache)
NSTREAM = (S - NCACHE * F) // FS  # 16 streamed sub-chunks
NCOL = NCACHE + NSTREAM           # stat columns

LAST_RESULTS = None  # BassKernelResults of the most recent run (for test.py)


def build_program(repeat=None, ncache=None, fs=None, stream_bufs=3,
                  cf=None, store_engine="sync", dma_scratch=2048):
    import concourse.bass as bass
    import concourse.tile as tile
    from concourse import bacc, mybir

    F_ = F if cf is None else cf
    NCACHE_ = NCACHE if ncache is None else ncache
    FS_ = FS if fs is None else fs
    NSTREAM_ = (S - NCACHE_ * F_) // FS_
    NCOL_ = NCACHE_ + NSTREAM_

    f32 = mybir.dt.float32
    AF = mybir.ActivationFunctionType
    AX = mybir.AxisListType

    nc = bacc.Bacc(
        "TRN2",
        target_bir_lowering=False,
        debug=False,
        enable_asserts=False,
        num_devices=NCORES,
        dynamic_dma_scratch_size=dma_scratch,
    )

    x_d = nc.dram_tensor("x", [BPC, C, H, W], f32, kind="ExternalInput")
    w1_d = nc.dram_tensor("w1", [RED, C], f32, kind="ExternalInput")
    b1_d = nc.dram_tensor("b1", [RED], f32, kind="ExternalInput")
    w2_d = nc.dram_tensor("w2", [C, RED], f32, kind="ExternalInput")
    b2_d = nc.dram_tensor("b2", [C], f32, kind="ExternalInput")
    out_d = nc.dram_tensor("out", [BPC, C, H, W], f32, kind="ExternalOutput")

    x2 = x_d.ap().rearrange("b c h w -> (b c) (h w)")    # [128, 65536]
    o2 = out_d.ap().rearrange("b c h w -> (b c) (h w)")  # [128, 65536]

    with tile.TileContext(nc) as tc:
        with (
            tc.tile_pool(name="cache", bufs=1) as cache_pool,
            tc.tile_pool(name="stream", bufs=stream_bufs) as stream_pool,
            tc.tile_pool(name="misc", bufs=1) as misc_pool,
            tc.tile_pool(name="psum", bufs=1, space="PSUM") as psum_pool,
        ):
            cach = [
                cache_pool.tile([P, F_], f32, tag=f"c{j}", name=f"c{j}")
                for j in range(NCACHE_)
            ]
            st = getattr(nc, store_engine)
            loop = tc.For_i(0, repeat, 1) if repeat else contextlib.nullcontext()
            with loop:
                # --- small persistent tiles -------------------------------
                stats_sum = misc_pool.tile([P, NCOL_], f32, tag="stats_sum")
                stats_max = misc_pool.tile([P, NCOL_], f32, tag="stats_max")

                w1T = misc_pool.tile([C, RED], f32, tag="w1T")   # [64, 16]
                nc.sync.dma_start(w1T[:], w1_d.ap().rearrange("r c -> c r"))
                w2T = misc_pool.tile([RED, C], f32, tag="w2T")   # [16, 64]
                nc.sync.dma_start(w2T[:], w2_d.ap().rearrange("c r -> r c"))
                b1t = misc_pool.tile([RED, 1], f32, tag="b1t")
                nc.sync.dma_start(b1t[:], b1_d.ap()[:, None])
                b2t2 = misc_pool.tile([C, 1], f32, tag="b2t2")
                nc.sync.dma_start(b2t2[:], b2_d.ap()[:, None])
                # the two MLP branches each add b2 -> fused bias is 2*b2
                nc.scalar.mul(b2t2[:], b2t2[:], 2.0)

                # chunk table: (stat col, spatial offset, width, cached tile)
                chunks = [(j, j * F_, F_, cach[j]) for j in range(NCACHE_)]
                chunks += [
                    (NCACHE_ + k, NCACHE_ * F_ + k * FS_, FS_, None)
                    for k in range(NSTREAM_)
                ]

                # --- pass 1: stream + pool --------------------------------
                for col, off, width, ct in chunks:
                    t = ct if ct is not None else stream_pool.tile(
                        [P, FS_], f32, name="s1", tag="stream"
                    )
                    nc.sync.dma_start(t[:, :width], x2[:, off:off + width])
                    # in-place Copy: the output is a no-op, accum_out gives
                    # the per-partition spatial sum on ScalarE
                    nc.scalar.activation(
                        t[:, :width], t[:, :width], AF.Copy,
                        accum_out=stats_sum[:, col:col + 1],
                    )
                    nc.vector.reduce_max(
                        stats_max[:, col:col + 1], t[:, :width], axis=AX.X
                    )

                # --- pass 2: MLP on pooled vectors ------------------------
                # fin[:, 0] = mean, fin[:, 1] = max   (p = b*64 + c)
                fin = misc_pool.tile([P, 2], f32, tag="fin")
                nc.vector.reduce_sum(fin[:, 0:1], stats_sum[:], axis=AX.X)
                nc.vector.reduce_max(fin[:, 1:2], stats_max[:], axis=AX.X)
                nc.scalar.mul(fin[:, 0:1], fin[:, 0:1], 1.0 / S)  # mean

                # SBUF->SBUF transpose: cols avg_b0, max_b0, avg_b1, max_b1
                vT = misc_pool.tile([C, 2 * BPC], f32, tag="vT")
                nc.sync.dma_start(vT[:, 0:2], fin[0:C, :])
                nc.sync.dma_start(vT[:, 2:4], fin[C:2 * C, :])

                h_ps = psum_pool.tile([RED, 2 * BPC], f32, tag="h_ps")
                nc.tensor.matmul(h_ps[:], w1T[:], vT[:], start=True, stop=True)
                h_sb = misc_pool.tile([RED, 2 * BPC], f32, tag="h_sb")
                nc.scalar.activation(
                    h_sb[:], h_ps[:], AF.Relu, bias=b1t[:], scale=1.0
                )

                # relu(h_avg) + relu(h_max), then one matmul for both branches
                h3 = h_sb.rearrange("r (b k) -> r b k", k=2)  # [16, 2, 2]
                hsum = misc_pool.tile([RED, BPC], f32, tag="hsum")
                nc.vector.tensor_add(hsum[:], h3[:, :, 0], h3[:, :, 1])

                y_ps = psum_pool.tile([C, BPC], f32, tag="y_ps")
                nc.tensor.matmul(y_ps[:], w2T[:], hsum[:], start=True, stop=True)
                y64 = misc_pool.tile([C, BPC], f32, tag="y64")
                nc.scalar.activation(
                    y64[:], y_ps[:], AF.Sigmoid, bias=b2t2[:], scale=1.0
                )

                # back to per-partition scalars [128, 1]
                y128 = misc_pool.tile([P, 1], f32, tag="y128")
                nc.sync.dma_start(y128[0:C, :], y64[:, 0:1])
                nc.sync.dma_start(y128[C:2 * C, :], y64[:, 1:2])

                # --- pass 3: rescale + store ------------------------------
                for col, off, width, ct in chunks:
                    if ct is not None:
                        t = ct
                    else:
                        t = stream_pool.tile([P, FS_], f32, name="s3", tag="stream")
                        nc.sync.dma_start(t[:, :width], x2[:, off:off + width])
                    nc.vector.tensor_scalar_mul(
                        t[:, :width], t[:, :width], y128[:]
                    )
                    st.dma_start(o2[:, off:off + width], t[:, :width])

    nc.compile()
    return nc


_NC_CACHE = None


def kernel(x, w1, b1, w2, b2, _trace=False):
    global LAST_RESULTS, _NC_CACHE
    from concourse.bass_utils import run_bass_kernel_spmd

    x = np.ascontiguousarray(x, dtype=np.float32)
    w1 = np.ascontiguousarray(w1, dtype=np.float32)
    b1 = np.ascontiguousarray(b1, dtype=np.float32)
    w2 = np.ascontiguousarray(w2, dtype=np.float32)
    b2 = np.ascontiguousarray(b2, dtype=np.float32)

    if _NC_CACHE is None:
        _NC_CACHE = build_program()
    nc = _NC_CACHE

    in_maps = [
        {
            "x": x[i * BPC:(i + 1) * BPC],
            "w1": w1,
            "b1": b1,
            "w2": w2,
            "b2": b2,
        }
        for i in range(NCORES)
    ]
    core_ids = list(range(NCORES))
    if _trace:
        # NTFF profiling (native runs); falls back when the profile hook
        # is unavailable (e.g. axon-tunneled cores).
        try:
            res = run_bass_kernel_spmd(nc, in_maps, core_ids, trace=True)
        except Exception:
            res = run_bass_kernel_spmd(nc, in_maps, core_ids, trace=False)
    else:
        res = run_bass_kernel_spmd(nc, in_maps, core_ids, trace=False)
    LAST_RESULTS = res
    out = np.concatenate([res.results[i]["out"] for i in range(NCORES)], axis=0)
    return out


# revision 22
# speedup vs baseline: 1.0069x; 1.0069x over previous
"""CALayer (channel attention) Trainium2 kernel.

x: [16, 64, 256, 256] f32.  y = sigmoid(mlp(avgpool(x)) + mlp(maxpool(x)));
out = x * y[:, :, None, None].

Sharding: data-parallel over batch, 2 samples per core on 8 cores. Per-core
view is [(2*64)=128 partitions, 65536 spatial].  Pass 1 streams chunks of
[128, 2048], computing the spatial sum on ScalarE (activation-Copy accum_out)
and the max on VectorE; most chunks stay resident in SBUF (and 6 PSUM banks
hold 3072 more columns).  Pass 2 runs the 64->16->64 MLP on TensorE with
zero-masked rhs halves so no SBUF->SBUF transpose DMAs sit on the critical
path.  Pass 3 rescales each chunk by the per-(b,c) sigmoid output and DMAs it
out; only the uncached tail re-reads HBM.
"""

import contextlib

import numpy as np

B, C, H, W = 16, 64, 256, 256
S = H * W            # 65536 spatial elements per (b, c)
RED = 16             # MLP hidden dim
NCORES = 8
BPC = B // NCORES    # batch per core = 2
P = BPC * C          # 128 partitions per core

F = 2048             # cached-chunk free-dim size (1 MiB per [128, F] f32 DMA)
NCACHE = 26          # chunks kept resident in SBUF between pass 1 and pass 3
FS = 1024            # streamed-chunk free-dim size (smaller -> more cache)

LAST_RESULTS = None  # BassKernelResults of the most recent run (for test.py)


def build_program(repeat=None, ncache=None, fs=None, stream_bufs=3,
                  cf=None, store_engine="sync", dma_scratch=2048,
                  mlp2=True, psum_cache=(1024, 1024, 1024), extra_cache=()):
    import concourse.bass as bass
    import concourse.tile as tile
    from concourse import bacc, mybir

    F_ = F if cf is None else cf
    NCACHE_ = NCACHE if ncache is None else ncache
    FS_ = FS if fs is None else fs
    psum_cache = tuple(psum_cache or ())
    extra_cache = tuple(extra_cache or ())
    assert all(w <= FS_ for w in psum_cache), "psum chunks load via stream tiles"
    cached_elems = NCACHE_ * F_ + sum(extra_cache) + sum(psum_cache)
    assert (S - cached_elems) % FS_ == 0
    NSTREAM_ = (S - cached_elems) // FS_

    f32 = mybir.dt.float32
    AF = mybir.ActivationFunctionType
    AX = mybir.AxisListType

    nc = bacc.Bacc(
        "TRN2",
        target_bir_lowering=False,
        debug=False,
        enable_asserts=False,
        num_devices=NCORES,
        dynamic_dma_scratch_size=dma_scratch,
    )

    x_d = nc.dram_tensor("x", [BPC, C, H, W], f32, kind="ExternalInput")
    w1_d = nc.dram_tensor("w1", [RED, C], f32, kind="ExternalInput")
    b1_d = nc.dram_tensor("b1", [RED], f32, kind="ExternalInput")
    w2_d = nc.dram_tensor("w2", [C, RED], f32, kind="ExternalInput")
    b2_d = nc.dram_tensor("b2", [C], f32, kind="ExternalInput")
    out_d = nc.dram_tensor("out", [BPC, C, H, W], f32, kind="ExternalOutput")

    x2 = x_d.ap().rearrange("b c h w -> (b c) (h w)")    # [128, 65536]
    o2 = out_d.ap().rearrange("b c h w -> (b c) (h w)")  # [128, 65536]

    with tile.TileContext(nc) as tc:
        with (
            tc.tile_pool(name="cache", bufs=1) as cache_pool,
            tc.tile_pool(name="stream", bufs=stream_bufs) as stream_pool,
            tc.tile_pool(name="misc", bufs=1) as misc_pool,
            tc.tile_pool(name="psum", bufs=1, space="PSUM") as psum_pool,
        ):
            # persistent SBUF cache tiles
            cach = [
                cache_pool.tile([P, F_], f32, tag=f"c{j}", name=f"c{j}")
                for j in range(NCACHE_)
            ]
            xcach = [
                cache_pool.tile([P, w], f32, tag=f"cx{j}", name=f"cx{j}")
                for j, w in enumerate(extra_cache)
            ]
            # persistent PSUM cache tiles (ScalarE parks, VectorE reads back)
            pcach = [
                psum_pool.tile([P, w], f32, tag=f"pc{j}", name=f"pc{j}")
                for j, w in enumerate(psum_cache)
            ]
            st = getattr(nc, store_engine)

            # chunk table: (kind, spatial offset, width, tile-or-None)
            chunks = []
            off = 0
            for t in cach + xcach:
                w = t.shape[-1]
                chunks.append(("sbuf", off, w, t))
                off += w
            for t in pcach:
                w = t.shape[-1]
                chunks.append(("psum", off, w, t))
                off += w
            for _ in range(NSTREAM_):
                chunks.append(("stream", off, FS_, None))
                off += FS_
            assert off == S
            NCOL_ = len(chunks)

            if mlp2:
                # zero-masked matmul rhs lives outside the loop: the two
                # ScalarE copies per iteration only touch their own halves
                mrhs = misc_pool.tile([P, 2 * BPC], f32, tag="mrhs")
                nc.vector.memset(mrhs[:], 0.0)

            loop = tc.For_i(0, repeat, 1) if repeat else contextlib.nullcontext()
            with loop:
                stats_sum = misc_pool.tile([P, NCOL_], f32, tag="stats_sum")
                stats_max = misc_pool.tile([P, NCOL_], f32, tag="stats_max")

                # --- weights (off the critical path) ----------------------
                b1t = misc_pool.tile([RED, 1], f32, tag="b1t")
                nc.sync.dma_start(b1t[:], b1_d.ap()[:, None])
                if mlp2:
                    # w1T duplicated across both partition halves: [128, 16]
                    w1T = misc_pool.tile([P, RED], f32, tag="w1T")
                    nc.sync.dma_start(w1T[0:C, :], w1_d.ap().rearrange("r c -> c r"))
                    nc.sync.dma_start(w1T[C:2 * C, :], w1_d.ap().rearrange("r c -> c r"))
                    # 2*b2 duplicated: [128, 1]
                    b2t2 = misc_pool.tile([P, 1], f32, tag="b2t2")
                    nc.sync.dma_start(b2t2[0:C, :], b2_d.ap()[:, None])
                    nc.sync.dma_start(b2t2[C:2 * C, :], b2_d.ap()[:, None])
                else:
                    w1T = misc_pool.tile([C, RED], f32, tag="w1T")
                    nc.sync.dma_start(w1T[:], w1_d.ap().rearrange("r c -> c r"))
                    b2t2 = misc_pool.tile([C, 1], f32, tag="b2t2")
                    nc.sync.dma_start(b2t2[:], b2_d.ap()[:, None])
                nc.scalar.mul(b2t2[:], b2t2[:], 2.0)
                w2T = misc_pool.tile([RED, C], f32, tag="w2T")   # [16, 64]
                nc.sync.dma_start(w2T[:], w2_d.ap().rearrange("c r -> r c"))

                # --- pass 1: stream + pool --------------------------------
                for col, (kind, off, width, ct) in enumerate(chunks):
                    if kind == "sbuf":
                        t = ct
                    else:
                        t = stream_pool.tile([P, FS_], f32, name="s1",
                                             tag="stream")
                    nc.sync.dma_start(t[:, :width], x2[:, off:off + width])
                    # in-place Copy: the output is a no-op, accum_out gives
                    # the per-partition spatial sum on ScalarE
                    nc.scalar.activation(
                        t[:, :width], t[:, :width], AF.Copy,
                        accum_out=stats_sum[:, col:col + 1],
                    )
                    nc.vector.reduce_max(
                        stats_max[:, col:col + 1], t[:, :width], axis=AX.X
                    )
                    if kind == "psum":
                        nc.scalar.copy(ct[:, :width], t[:, :width])

                # --- pass 2: MLP on pooled vectors ------------------------
                # fin[:, 0] = mean, fin[:, 1] = max   (p = b*64 + c)
                fin = misc_pool.tile([P, 2], f32, tag="fin")
                nc.vector.reduce_sum(fin[:, 0:1], stats_sum[:], axis=AX.X)
                nc.vector.reduce_max(fin[:, 1:2], stats_max[:], axis=AX.X)
                nc.scalar.mul(fin[:, 0:1], fin[:, 0:1], 1.0 / S)  # mean

                if mlp2:
                    # lane-aligned ScalarE copies, no SBUF->SBUF DMA: rows
                    # 0-63 fill cols 0:2 (sample b0), rows 64-127 cols 2:4
                    nc.scalar.copy(mrhs[0:C, 0:2], fin[0:C, :])
                    nc.scalar.copy(mrhs[C:2 * C, 2:4], fin[C:2 * C, :])
                    # contraction over all 128 partitions; the zero-masked
                    # halves contribute nothing -> per-sample dot products
                    h_ps = psum_pool.tile([RED, 2 * BPC], f32, tag="h_ps")
                    nc.tensor.matmul(h_ps[:], w1T[:], mrhs[:],
                                     start=True, stop=True)
                else:
                    vT = misc_pool.tile([C, 2 * BPC], f32, tag="vT")
                    nc.sync.dma_start(vT[:, 0:2], fin[0:C, :])
                    nc.sync.dma_start(vT[:, 2:4], fin[C:2 * C, :])
                    h_ps = psum_pool.tile([RED, 2 * BPC], f32, tag="h_ps")
                    nc.tensor.matmul(h_ps[:], w1T[:], vT[:],
                                     start=True, stop=True)

                # cols are (b0: avg,max | b1: avg,max)
                h_sb = misc_pool.tile([RED, 2 * BPC], f32, tag="h_sb")
                nc.scalar.activation(
                    h_sb[:], h_ps[:], AF.Relu, bias=b1t[:], scale=1.0
                )
                # relu(h_avg) + relu(h_max), one second-layer matmul per sample
                h3 = h_sb.rearrange("r (b k) -> r b k", k=2)  # [16, 2, 2]
                hsum = misc_pool.tile([RED, BPC], f32, tag="hsum")
                nc.vector.tensor_add(hsum[:], h3[:, :, 0], h3[:, :, 1])

                if mlp2:
                    # y as [128, 1] directly: two matmuls into disjoint
                    # partition halves of one PSUM tile
                    y_ps = psum_pool.tile([P, 1], f32, tag="y_ps")
                    nc.tensor.matmul(y_ps[0:C, :], w2T[:], hsum[:, 0:1],
                                     start=True, stop=True)
                    nc.tensor.matmul(y_ps[C:2 * C, :], w2T[:], hsum[:, 1:2],
                                     start=True, stop=True)
                    y128 = misc_pool.tile([P, 1], f32, tag="y128")
                    nc.scalar.activation(
                        y128[:], y_ps[:], AF.Sigmoid, bias=b2t2[:], scale=1.0
                    )
                else:
                    y_ps = psum_pool.tile([C, BPC], f32, tag="y_ps")
                    nc.tensor.matmul(y_ps[:], w2T[:], hsum[:],
                                     start=True, stop=True)
                    y64 = misc_pool.tile([C, BPC], f32, tag="y64")
                    nc.scalar.activation(
                        y64[:], y_ps[:], AF.Sigmoid, bias=b2t2[:], scale=1.0
                    )
                    y128 = misc_pool.tile([P, 1], f32, tag="y128")
                    nc.sync.dma_start(y128[0:C, :], y64[:, 0:1])
                    nc.sync.dma_start(y128[C:2 * C, :], y64[:, 1:2])

                # --- pass 3: rescale + store ------------------------------
                for kind, off, width, ct in chunks:
                    if kind == "sbuf":
                        t = ct
                        nc.vector.tensor_scalar_mul(
                            t[:, :width], t[:, :width], y128[:]
                        )
                    elif kind == "psum":
                        t = stream_pool.tile([P, FS_], f32, name="s3p",
                                             tag="stream")
                        # VectorE reads the parked PSUM data back to SBUF
                        nc.vector.tensor_scalar_mul(
                            t[:, :width], ct[:, :width], y128[:]
                        )
                    else:
                        t = stream_pool.tile([P, FS_], f32, name="s3",
                                             tag="stream")
                        nc.sync.dma_start(t[:, :width], x2[:, off:off + width])
                        nc.vector.tensor_scalar_mul(
                            t[:, :width], t[:, :width], y128[:]
                        )
                    st.dma_start(o2[:, off:off + width], t[:, :width])

    nc.compile()
    return nc


_NC_CACHE = None


def kernel(x, w1, b1, w2, b2, _trace=False):
    global LAST_RESULTS, _NC_CACHE
    from concourse.bass_utils import run_bass_kernel_spmd

    x = np.ascontiguousarray(x, dtype=np.float32)
    w1 = np.ascontiguousarray(w1, dtype=np.float32)
    b1 = np.ascontiguousarray(b1, dtype=np.float32)
    w2 = np.ascontiguousarray(w2, dtype=np.float32)
    b2 = np.ascontiguousarray(b2, dtype=np.float32)

    if _NC_CACHE is None:
        _NC_CACHE = build_program()
    nc = _NC_CACHE

    in_maps = [
        {
            "x": x[i * BPC:(i + 1) * BPC],
            "w1": w1,
            "b1": b1,
            "w2": w2,
            "b2": b2,
        }
        for i in range(NCORES)
    ]
    core_ids = list(range(NCORES))
    if _trace:
        # NTFF profiling (native runs); falls back when the profile hook
        # is unavailable (e.g. axon-tunneled cores).
        try:
            res = run_bass_kernel_spmd(nc, in_maps, core_ids, trace=True)
        except Exception:
            res = run_bass_kernel_spmd(nc, in_maps, core_ids, trace=False)
    else:
        res = run_bass_kernel_spmd(nc, in_maps, core_ids, trace=False)
    LAST_RESULTS = res
    out = np.concatenate([res.results[i]["out"] for i in range(NCORES)], axis=0)
    return out


# revision 24
# speedup vs baseline: 1.0276x; 1.0205x over previous
"""CALayer (channel attention) Trainium2 kernel.

x: [16, 64, 256, 256] f32.  y = sigmoid(mlp(avgpool(x)) + mlp(maxpool(x)));
out = x * y[:, :, None, None].

Sharding: data-parallel over batch, 2 samples per core on 8 cores. Per-core
view is [(2*64)=128 partitions, 65536 spatial].  Pass 1 streams chunks of
[128, 2048], computing the spatial sum on ScalarE (activation-Copy accum_out)
and the max on VectorE; most chunks stay resident in SBUF (and 6 PSUM banks
hold 3072 more columns).  Pass 2 runs the 64->16->64 MLP on TensorE with
zero-masked rhs halves so no SBUF->SBUF transpose DMAs sit on the critical
path.  Pass 3 rescales each chunk by the per-(b,c) sigmoid output and DMAs it
out; only the uncached tail re-reads HBM.
"""

import contextlib

import numpy as np

B, C, H, W = 16, 64, 256, 256
S = H * W            # 65536 spatial elements per (b, c)
RED = 16             # MLP hidden dim
NCORES = 8
BPC = B // NCORES    # batch per core = 2
P = BPC * C          # 128 partitions per core

F = 2048             # cached-chunk free-dim size (1 MiB per [128, F] f32 DMA)
NCACHE = 26          # chunks kept resident in SBUF between pass 1 and pass 3
FS = 1024            # streamed-chunk free-dim size (smaller -> more cache)

LAST_RESULTS = None  # BassKernelResults of the most recent run (for test.py)


def build_program(repeat=None, ncache=None, fs=None, stream_bufs=3,
                  cf=None, store_engine="sync", dma_scratch=2048,
                  mlp2=True, psum_cache=(1024, 1024, 1024), extra_cache=(),
                  stream_first=True):
    import concourse.bass as bass
    import concourse.tile as tile
    from concourse import bacc, mybir

    F_ = F if cf is None else cf
    NCACHE_ = NCACHE if ncache is None else ncache
    FS_ = FS if fs is None else fs
    psum_cache = tuple(psum_cache or ())
    extra_cache = tuple(extra_cache or ())
    assert all(w <= FS_ for w in psum_cache), "psum chunks load via stream tiles"
    cached_elems = NCACHE_ * F_ + sum(extra_cache) + sum(psum_cache)
    assert (S - cached_elems) % FS_ == 0
    NSTREAM_ = (S - cached_elems) // FS_

    f32 = mybir.dt.float32
    AF = mybir.ActivationFunctionType
    AX = mybir.AxisListType

    nc = bacc.Bacc(
        "TRN2",
        target_bir_lowering=False,
        debug=False,
        enable_asserts=False,
        num_devices=NCORES,
        dynamic_dma_scratch_size=dma_scratch,
    )

    x_d = nc.dram_tensor("x", [BPC, C, H, W], f32, kind="ExternalInput")
    w1_d = nc.dram_tensor("w1", [RED, C], f32, kind="ExternalInput")
    b1_d = nc.dram_tensor("b1", [RED], f32, kind="ExternalInput")
    w2_d = nc.dram_tensor("w2", [C, RED], f32, kind="ExternalInput")
    b2_d = nc.dram_tensor("b2", [C], f32, kind="ExternalInput")
    out_d = nc.dram_tensor("out", [BPC, C, H, W], f32, kind="ExternalOutput")

    x2 = x_d.ap().rearrange("b c h w -> (b c) (h w)")    # [128, 65536]
    o2 = out_d.ap().rearrange("b c h w -> (b c) (h w)")  # [128, 65536]

    with tile.TileContext(nc) as tc:
        with (
            tc.tile_pool(name="cache", bufs=1) as cache_pool,
            tc.tile_pool(name="stream", bufs=stream_bufs) as stream_pool,
            tc.tile_pool(name="misc", bufs=1) as misc_pool,
            tc.tile_pool(name="psum", bufs=1, space="PSUM") as psum_pool,
        ):
            # persistent SBUF cache tiles
            cach = [
                cache_pool.tile([P, F_], f32, tag=f"c{j}", name=f"c{j}")
                for j in range(NCACHE_)
            ]
            xcach = [
                cache_pool.tile([P, w], f32, tag=f"cx{j}", name=f"cx{j}")
                for j, w in enumerate(extra_cache)
            ]
            # persistent PSUM cache tiles (ScalarE parks, VectorE reads back)
            pcach = [
                psum_pool.tile([P, w], f32, tag=f"pc{j}", name=f"pc{j}")
                for j, w in enumerate(psum_cache)
            ]
            st = getattr(nc, store_engine)

            # chunk table: (kind, spatial offset, width, tile-or-None).
            # stream_first puts the slot-rotating chunks early so their
            # pass-1 slots free early and pass-3 re-reads hoist into the
            # pass-1 read burst; pass 3 then ends as a pure write stream.
            chunks = []
            off = 0
            for t in cach + xcach:
                w = t.shape[-1]
                chunks.append(("sbuf", off, w, t))
                off += w
            for t in pcach:
                w = t.shape[-1]
                chunks.append(("psum", off, w, t))
                off += w
            for _ in range(NSTREAM_):
                chunks.append(("stream", off, FS_, None))
                off += FS_
            assert off == S
            if stream_first:
                chunks = (
                    [c for c in chunks if c[0] != "sbuf"]
                    + [c for c in chunks if c[0] == "sbuf"]
                )
            NCOL_ = len(chunks)

            if mlp2:
                # zero-masked matmul rhs lives outside the loop: the two
                # ScalarE copies per iteration only touch their own halves
                mrhs = misc_pool.tile([P, 2 * BPC], f32, tag="mrhs")
                nc.vector.memset(mrhs[:], 0.0)

            loop = tc.For_i(0, repeat, 1) if repeat else contextlib.nullcontext()
            with loop:
                stats_sum = misc_pool.tile([P, NCOL_], f32, tag="stats_sum")
                stats_max = misc_pool.tile([P, NCOL_], f32, tag="stats_max")

                # --- weights (off the critical path) ----------------------
                b1t = misc_pool.tile([RED, 1], f32, tag="b1t")
                nc.sync.dma_start(b1t[:], b1_d.ap()[:, None])
                if mlp2:
                    # w1T duplicated across both partition halves: [128, 16]
                    w1T = misc_pool.tile([P, RED], f32, tag="w1T")
                    nc.sync.dma_start(w1T[0:C, :], w1_d.ap().rearrange("r c -> c r"))
                    nc.sync.dma_start(w1T[C:2 * C, :], w1_d.ap().rearrange("r c -> c r"))
                    # 2*b2 duplicated: [128, 1]
                    b2t2 = misc_pool.tile([P, 1], f32, tag="b2t2")
                    nc.sync.dma_start(b2t2[0:C, :], b2_d.ap()[:, None])
                    nc.sync.dma_start(b2t2[C:2 * C, :], b2_d.ap()[:, None])
                else:
                    w1T = misc_pool.tile([C, RED], f32, tag="w1T")
                    nc.sync.dma_start(w1T[:], w1_d.ap().rearrange("r c -> c r"))
                    b2t2 = misc_pool.tile([C, 1], f32, tag="b2t2")
                    nc.sync.dma_start(b2t2[:], b2_d.ap()[:, None])
                nc.scalar.mul(b2t2[:], b2t2[:], 2.0)
                w2T = misc_pool.tile([RED, C], f32, tag="w2T")   # [16, 64]
                nc.sync.dma_start(w2T[:], w2_d.ap().rearrange("c r -> r c"))

                # --- pass 1: stream + pool --------------------------------
                for col, (kind, off, width, ct) in enumerate(chunks):
                    if kind == "sbuf":
                        t = ct
                    else:
                        t = stream_pool.tile([P, FS_], f32, name="s1",
                                             tag="stream")
                    nc.sync.dma_start(t[:, :width], x2[:, off:off + width])
                    # in-place Copy: the output is a no-op, accum_out gives
                    # the per-partition spatial sum on ScalarE
                    nc.scalar.activation(
                        t[:, :width], t[:, :width], AF.Copy,
                        accum_out=stats_sum[:, col:col + 1],
                    )
                    nc.vector.reduce_max(
                        stats_max[:, col:col + 1], t[:, :width], axis=AX.X
                    )
                    if kind == "psum":
                        nc.scalar.copy(ct[:, :width], t[:, :width])

                # --- pass 2: MLP on pooled vectors ------------------------
                # fin[:, 0] = mean, fin[:, 1] = max   (p = b*64 + c)
                fin = misc_pool.tile([P, 2], f32, tag="fin")
                nc.vector.reduce_sum(fin[:, 0:1], stats_sum[:], axis=AX.X)
                nc.vector.reduce_max(fin[:, 1:2], stats_max[:], axis=AX.X)
                nc.scalar.mul(fin[:, 0:1], fin[:, 0:1], 1.0 / S)  # mean

                if mlp2:
                    # lane-aligned ScalarE copies, no SBUF->SBUF DMA: rows
                    # 0-63 fill cols 0:2 (sample b0), rows 64-127 cols 2:4
                    nc.scalar.copy(mrhs[0:C, 0:2], fin[0:C, :])
                    nc.scalar.copy(mrhs[C:2 * C, 2:4], fin[C:2 * C, :])
                    # contraction over all 128 partitions; the zero-masked
                    # halves contribute nothing -> per-sample dot products
                    h_ps = psum_pool.tile([RED, 2 * BPC], f32, tag="h_ps")
                    nc.tensor.matmul(h_ps[:], w1T[:], mrhs[:],
                                     start=True, stop=True)
                else:
                    vT = misc_pool.tile([C, 2 * BPC], f32, tag="vT")
                    nc.sync.dma_start(vT[:, 0:2], fin[0:C, :])
                    nc.sync.dma_start(vT[:, 2:4], fin[C:2 * C, :])
                    h_ps = psum_pool.tile([RED, 2 * BPC], f32, tag="h_ps")
                    nc.tensor.matmul(h_ps[:], w1T[:], vT[:],
                                     start=True, stop=True)

                # cols are (b0: avg,max | b1: avg,max)
                h_sb = misc_pool.tile([RED, 2 * BPC], f32, tag="h_sb")
                nc.scalar.activation(
                    h_sb[:], h_ps[:], AF.Relu, bias=b1t[:], scale=1.0
                )
                # relu(h_avg) + relu(h_max), one second-layer matmul per sample
                h3 = h_sb.rearrange("r (b k) -> r b k", k=2)  # [16, 2, 2]
                hsum = misc_pool.tile([RED, BPC], f32, tag="hsum")
                nc.vector.tensor_add(hsum[:], h3[:, :, 0], h3[:, :, 1])

                if mlp2:
                    # y as [128, 1] directly: two matmuls into disjoint
                    # partition halves of one PSUM tile
                    y_ps = psum_pool.tile([P, 1], f32, tag="y_ps")
                    nc.tensor.matmul(y_ps[0:C, :], w2T[:], hsum[:, 0:1],
                                     start=True, stop=True)
                    nc.tensor.matmul(y_ps[C:2 * C, :], w2T[:], hsum[:, 1:2],
                                     start=True, stop=True)
                    y128 = misc_pool.tile([P, 1], f32, tag="y128")
                    nc.scalar.activation(
                        y128[:], y_ps[:], AF.Sigmoid, bias=b2t2[:], scale=1.0
                    )
                else:
                    y_ps = psum_pool.tile([C, BPC], f32, tag="y_ps")
                    nc.tensor.matmul(y_ps[:], w2T[:], hsum[:],
                                     start=True, stop=True)
                    y64 = misc_pool.tile([C, BPC], f32, tag="y64")
                    nc.scalar.activation(
                        y64[:], y_ps[:], AF.Sigmoid, bias=b2t2[:], scale=1.0
                    )
                    y128 = misc_pool.tile([P, 1], f32, tag="y128")
                    nc.sync.dma_start(y128[0:C, :], y64[:, 0:1])
                    nc.sync.dma_start(y128[C:2 * C, :], y64[:, 1:2])

                # --- pass 3: rescale + store ------------------------------
                for kind, off, width, ct in chunks:
                    if kind == "sbuf":
                        t = ct
                        nc.vector.tensor_scalar_mul(
                            t[:, :width], t[:, :width], y128[:]
                        )
                    elif kind == "psum":
                        t = stream_pool.tile([P, FS_], f32, name="s3p",
                                             tag="stream")
                        # VectorE reads the parked PSUM data back to SBUF
                        nc.vector.tensor_scalar_mul(
                            t[:, :width], ct[:, :width], y128[:]
                        )
                    else:
                        t = stream_pool.tile([P, FS_], f32, name="s3",
                                             tag="stream")
                        nc.sync.dma_start(t[:, :width], x2[:, off:off + width])
                        nc.vector.tensor_scalar_mul(
                            t[:, :width], t[:, :width], y128[:]
                        )
                    st.dma_start(o2[:, off:off + width], t[:, :width])

    nc.compile()
    return nc


_NC_CACHE = None


def kernel(x, w1, b1, w2, b2, _trace=False):
    global LAST_RESULTS, _NC_CACHE
    from concourse.bass_utils import run_bass_kernel_spmd

    x = np.ascontiguousarray(x, dtype=np.float32)
    w1 = np.ascontiguousarray(w1, dtype=np.float32)
    b1 = np.ascontiguousarray(b1, dtype=np.float32)
    w2 = np.ascontiguousarray(w2, dtype=np.float32)
    b2 = np.ascontiguousarray(b2, dtype=np.float32)

    if _NC_CACHE is None:
        _NC_CACHE = build_program()
    nc = _NC_CACHE

    in_maps = [
        {
            "x": x[i * BPC:(i + 1) * BPC],
            "w1": w1,
            "b1": b1,
            "w2": w2,
            "b2": b2,
        }
        for i in range(NCORES)
    ]
    core_ids = list(range(NCORES))
    if _trace:
        # NTFF profiling (native runs); falls back when the profile hook
        # is unavailable (e.g. axon-tunneled cores).
        try:
            res = run_bass_kernel_spmd(nc, in_maps, core_ids, trace=True)
        except Exception:
            res = run_bass_kernel_spmd(nc, in_maps, core_ids, trace=False)
    else:
        res = run_bass_kernel_spmd(nc, in_maps, core_ids, trace=False)
    LAST_RESULTS = res
    out = np.concatenate([res.results[i]["out"] for i in range(NCORES)], axis=0)
    return out
